# revision 26
# baseline (speedup 1.0000x reference)
"""Trainium2 Bass kernel for AdaptiveMixtureOfExperts (top-2 SwiGLU MoE).

Strategy (expert-parallel, quarter-FF load balancing):
  - Host computes the tiny router (x @ Wr, top-2, softmax) with jax-on-CPU ops
    that bit-match the reference, then shards tokens by routed expert.
  - Each expert's FFN is split into 4 quarters along D_FF.  Experts are ranked
    by token count; rank pair (2p, 2p+1) forms section-position p, so each of
    the 8 cores runs 4 quarter-FFN sections (one per position).  Per-position
    token capacity = pad8(max count of the pair), giving per-core work within
    ~1.4% of the mean.
        hT = W1q.T @ xqT            (ff on partitions, tokens on free dim)
        uT = (a + b1a) * silu(g + b1g)
        yT_partial = W2q.T @ uT
  - Host sums the 4 quarter contributions per expert, adds b2, applies the
    top-2 combine weights, and scatter-adds into the full [B, S, D] output.

  DMA layout: every transfer is a contiguous 2D block with multi-KB rows
  (DMA descriptor feed is the early-phase bottleneck; thin-row strided DMAs
  feed ~4x slower).  Host packs token buffers tile-major ([P, K1*C], token
  tiles consecutive), W1 chunk-major ([P, NBLK/2 chunks of K1 x 2P cols]),
  and the y output tile-major ([P, NO*C]).  All inputs ride qSP in exact
  consumption order; y outputs and the tiny b1 vectors ride qACT so neither
  queue head-of-line blocks the other's dependencies.  Section 2/3 token
  loads (which wait on xg buffer reuse) precede their weights, idling the
  shared DMA pool exactly when mm2-s0/s1's outputs need to drain.

Shapes hardcoded for the problem instance:
  x:[2,2048,1024] f32, Wr:[1024,8], temp:[1], W1:[8,1024,4096], b1:[8,4096],
  W2:[8,2048,1024], b2:[8,1024].  TOP_K=2, 8 experts on 8 cores.
"""

import os

import numpy as np
import ml_dtypes

D_MODEL = 1024
D_FF = 2048
NUM_EXPERTS = 8
TOP_K = 2
P = 128          # partitions
NT = 512         # token tile (moving free dim per matmul)
T0 = 128         # first token tile of section 0 (small for an early start)
N_CORES = 8
NSEC = 4         # sections per core (one expert-quarter each)
FQ = D_FF // NSEC            # 512: ff quarter
K1 = D_MODEL // P            # 8 k-tiles for matmul1
K2 = FQ // P                 # 4 k-tiles for matmul2
NBLK = 2 * FQ // P           # 8 ff blocks per section (a/g interleaved)
NO = D_MODEL // P            # 8 output blocks of yT
WCH = K1 * 2 * P             # 2048: w1 columns per i-chunk in chunk-major
WARMUP = 13                  # PE warmup matmuls (cover preamble+first DMA)

_NC_CACHE = {}
LAST_RESULTS = None  # test harness introspection


def _tiles(C, first=False):
    """Token tile (offset, size) list.  With first=True the leading tile is
    halved so compute can start while the DMA ring is still slow; a 256-token
    tile's per-i compute (~1.7us) roughly matches one w1 chunk's DMA time."""
    cuts = [0]
    if first and 2 * T0 < C:
        cuts.append(2 * T0)
    while cuts[-1] < C:
        cuts.append(min(cuts[-1] + NT, C))
    return list(zip(cuts[:-1], (b - a for a, b in zip(cuts[:-1], cuts[1:]))))


def _build_nc(CS, use_silu: bool = True):
    """Per-core Bass graph: NSEC quarter-FF FFN sections of CS[s] tokens."""
    import concourse.mybir as mybir
    import concourse.tile as tile
    from concourse import bacc

    f32 = mybir.dt.float32
    bf16 = mybir.dt.bfloat16
    AF = mybir.ActivationFunctionType

    nc = bacc.Bacc()
    xq = {}
    w1 = {}
    w2 = {}
    b1t = {}
    outp = {}
    for s, C in enumerate(CS):
        xq[s] = nc.declare_dram_parameter(f"xq{s}", [P, K1 * C], bf16, isOutput=False)
        w1[s] = nc.declare_dram_parameter(f"w1{s}", [P, NBLK * K1 * P], bf16, isOutput=False)
        w2[s] = nc.declare_dram_parameter(f"w2{s}", [P, K2 * D_MODEL], bf16, isOutput=False)
        b1t[s] = nc.declare_dram_parameter(f"b1t{s}", [P, NBLK], f32, isOutput=False)
        # partial y without b2 (host adds the bias once per expert), bf16 to
        # halve output DMA bytes; tile-major [P, NO*C]
        outp[s] = nc.declare_dram_parameter(f"out{s}", [P, NO * C], bf16, isOutput=True)

    CMAX = max(CS)

    with tile.TileContext(nc) as tc:
        with (
            tc.tile_pool(name="weights", bufs=1) as wpool,
            tc.tile_pool(name="acts", bufs=2) as upool,
            tc.tile_pool(name="epilogue", bufs=4) as epool,
            tc.tile_pool(name="ps", bufs=8, space="PSUM") as ps_pool,
        ):
            b1_sb = {}
            for s in range(NSEC):
                b1_sb[s] = wpool.tile([P, NBLK], f32, name=f"b1_sb{s}", tag=f"b1{s}")

            # weights resident per section; tokens double-buffered across
            # sections (section s+2's load waits for section s's reads)
            w1_sb = {}
            w2_sb = {}
            xg_sb = {}
            for s in range(NSEC):
                w1_sb[s] = wpool.tile([P, NBLK * K1 * P], bf16, name=f"w1_sb{s}",
                                      tag=f"w1{s}")
                w2_sb[s] = wpool.tile([P, K2 * D_MODEL], bf16, name=f"w2_sb{s}",
                                      tag=f"w2{s}")
                xg_sb[s] = upool.tile([P, K1 * CMAX], bf16, name=f"xg_sb{s}",
                                      tag="xg", bufs=2)

            # PE warmup: dummy matmuls on a zeroed tile keep the PE busy (and
            # open the HAM clock gate to 2.4 GHz) until the first input DMAs
            # land (~6us fixed preamble + first chunks).
            warm = wpool.tile([P, NT], bf16, name="warm")
            nc.gpsimd.memset(warm[:], 0.0)
            ps_w = ps_pool.tile([P, NT], f32, name="ps_warm", tag="ps")
            for _ in range(WARMUP):
                nc.tensor.matmul(ps_w[:], warm[:, :P], warm[:], start=True, stop=True)
            # gate matmuls: read (uninitialized) section-1 tiles so their
            # DMAs carry a write-after-read wait until warmup completes.
            # With all 8 cores hammering the shared HBM path at launch, this
            # halves the aggregate early demand (only section-0 streams),
            # so every core's critical first chunks land sooner.  The values
            # read are garbage into a dead PSUM tile -- never consumed.
            nc.tensor.matmul(ps_w[:], w1_sb[1][:, :P], xg_sb[1][:, :NT],
                             start=True, stop=True)
            nc.tensor.matmul(ps_w[:], w2_sb[1][:, :P], warm[:],
                             start=True, stop=True)

            # ---- tiny early inputs on qACT (biases) ----
            for s in range(NSEC):
                nc.scalar.dma_start(out=b1_sb[s][:], in_=b1t[s][:])

            # ---- bulk inputs on qSP in exact PE consumption order ----
            # (the shared DMA pool caps at ~220GB/s; both queues draw from
            # it, so the early phase is paced purely by bytes-before-need)
            def emit_w1(s, i0, i1):
                nc.sync.dma_start(
                    out=w1_sb[s][:, i0 * WCH:i1 * WCH],
                    in_=w1[s][:, i0 * WCH:i1 * WCH])

            def emit_xg(s, off, end):
                nc.sync.dma_start(
                    out=xg_sb[s][:, K1 * off:K1 * end],
                    in_=xq[s][:, K1 * off:K1 * end])

            # section 0: mm1 runs t-outer, so tile 0 needs w1 i-chunks in
            # order (0.5MB steps) and later tiles need tokens -- emit in that
            # exact consumption order so each wait is short and the PE never
            # idles past the HAM re-throttle window
            s0_tiles = _tiles(CS[0], first=True)
            emit_w1(0, 0, 1)
            emit_xg(0, 0, s0_tiles[0][1])
            emit_w1(0, 1, 2)
            emit_xg(0, s0_tiles[1][0], s0_tiles[1][0] + s0_tiles[1][1])
            emit_w1(0, 2, 3)
            emit_w1(0, 3, K2)
            for off, Nt in s0_tiles[2:]:
                emit_xg(0, off, off + Nt)
            nc.sync.dma_start(out=w2_sb[0][:], in_=w2[0][:])
            # section 1: w1 head, tokens, w1 tail, w2
            emit_w1(1, 0, 1)
            emit_xg(1, 0, CS[1])
            emit_w1(1, 1, K2)
            nc.sync.dma_start(out=w2_sb[1][:], in_=w2[1][:])
            # sections 2/3: token loads reuse the xg buffers of sections 0/1,
            # so they wait on mm1-s0/s1's last reads.  Putting them FIRST
            # deliberately head-of-line blocks the input queue at ~the end of
            # mm1-s0/s1 -- that idles the shared DMA-engine pool exactly when
            # the y outputs of mm2-s0/s1 need it, then resumes with s2/s3
            # weights (still >30us before their matmuls need them)
            emit_xg(2, 0, CS[2])
            emit_w1(2, 0, K2)
            nc.sync.dma_start(out=w2_sb[2][:], in_=w2[2][:])
            emit_xg(3, 0, CS[3])
            emit_w1(3, 0, K2)
            nc.sync.dma_start(out=w2_sb[3][:], in_=w2[3][:])

            # ---- main loops ----
            # w1_sb chunk-major: i-block i at [i*WCH, (i+1)*WCH), inside it
            # k at [k*2P, k*2P+2P), a then g.  xg_sb tile-major: token tile
            # at [K1*off, K1*(off+Nt)), inside it k at [k*Nt, (k+1)*Nt).
            uT = {}

            def emit_mm1(s, staggered=False):
                tl = _tiles(CS[s], first=staggered)
                for t in range(len(tl)):
                    uT[(s, t)] = upool.tile(
                        [P, K2, NT], bf16, name=f"uT{s}{t}", tag="uT", bufs=4)
                for t, (off, Nt) in enumerate(tl):
                    for i in range(K2):
                        xbase = K1 * off
                        wbase = i * WCH
                        ps_a = ps_pool.tile(
                            [P, NT], f32, name=f"psa{s}{t}_{i}", tag="ps")
                        for k in range(K1):
                            nc.tensor.matmul(
                                ps_a[:, :Nt],
                                w1_sb[s][:, wbase + k * 2 * P:wbase + k * 2 * P + P],
                                xg_sb[s][:, xbase + k * Nt:xbase + (k + 1) * Nt],
                                start=(k == 0),
                                stop=(k == K1 - 1),
                            )
                        ps_g = ps_pool.tile(
                            [P, NT], f32, name=f"psg{s}{t}_{i}", tag="ps")
                        for k in range(K1):
                            nc.tensor.matmul(
                                ps_g[:, :Nt],
                                w1_sb[s][:, wbase + k * 2 * P + P:wbase + (k + 1) * 2 * P],
                                xg_sb[s][:, xbase + k * Nt:xbase + (k + 1) * Nt],
                                start=(k == 0),
                                stop=(k == K1 - 1),
                            )
                        a_t = epool.tile([P, NT], bf16, name=f"a{s}{t}_{i}",
                                         tag="a")
                        nc.scalar.activation(
                            a_t[:, :Nt], ps_a[:, :Nt], AF.Identity,
                            bias=b1_sb[s][:, 2 * i:2 * i + 1],
                        )
                        g_t = epool.tile([P, NT], bf16, name=f"g{s}{t}_{i}",
                                         tag="g")
                        if use_silu:
                            nc.scalar.activation(
                                g_t[:, :Nt], ps_g[:, :Nt], AF.Silu,
                                bias=b1_sb[s][:, 2 * i + 1:2 * i + 2],
                            )
                        else:
                            s_t = epool.tile(
                                [P, NT], bf16, name=f"s{s}{t}_{i}", tag="s")
                            nc.scalar.activation(
                                s_t[:, :Nt], ps_g[:, :Nt], AF.Sigmoid,
                                bias=b1_sb[s][:, 2 * i + 1:2 * i + 2],
                            )
                            gb_t = epool.tile(
                                [P, NT], bf16, name=f"gb{s}{t}_{i}", tag="gb")
                            nc.scalar.activation(
                                gb_t[:, :Nt], ps_g[:, :Nt], AF.Identity,
                                bias=b1_sb[s][:, 2 * i + 1:2 * i + 2],
                            )
                            nc.vector.tensor_mul(
                                g_t[:, :Nt], gb_t[:, :Nt], s_t[:, :Nt])
                        nc.vector.tensor_mul(
                            uT[(s, t)][:, i, :Nt], a_t[:, :Nt], g_t[:, :Nt])

            def emit_mm2(s, staggered=False, fine_tail=False):
                tl = _tiles(CS[s], first=staggered)
                for t, (off, Nt) in enumerate(tl):
                    last_tile = fine_tail and t == len(tl) - 1
                    ybase = NO * off
                    y_t = epool.tile([P, NO * NT], bf16, name=f"y{s}{t}",
                                     tag="y", bufs=4)
                    for m in range(NO):
                        ps_y = ps_pool.tile(
                            [P, NT], f32, name=f"psy{s}{t}_{m}", tag="ps")
                        for k in range(K2):
                            nc.tensor.matmul(
                                ps_y[:, :Nt],
                                w2_sb[s][:, k * D_MODEL + m * P:k * D_MODEL + (m + 1) * P],
                                uT[(s, t)][:, k, :Nt],
                                start=(k == 0),
                                stop=(k == K2 - 1),
                            )
                        # psum drain on DVE (idle), output via qACT (y DMAs +
                        # b1 are the only users, so the input stream on qSP is
                        # never blocked and y drains promptly)
                        nc.vector.tensor_copy(y_t[:, m * Nt:(m + 1) * Nt],
                                              ps_y[:, :Nt])
                        if last_tile:
                            nc.scalar.dma_start(
                                out=outp[s][:, ybase + m * Nt:ybase + (m + 1) * Nt],
                                in_=y_t[:, m * Nt:(m + 1) * Nt],
                            )
                        elif m == NO // 2 - 1:
                            nc.scalar.dma_start(
                                out=outp[s][:, ybase:ybase + (NO // 2) * Nt],
                                in_=y_t[:, :(NO // 2) * Nt],
                            )
                        elif m == NO - 1:
                            nc.scalar.dma_start(
                                out=outp[s][:, ybase + (NO // 2) * Nt:ybase + NO * Nt],
                                in_=y_t[:, (NO // 2) * Nt:NO * Nt],
                            )

            for s in range(NSEC):
                emit_mm1(s, staggered=(s == 0))
                emit_mm2(s, staggered=(s == 0), fine_tail=(s == NSEC - 1))

    nc.compile()
    return nc


def _route_tokens(xf, Wr, temp):
    """Bit-match the reference's router on CPU jax: logits, top-2, softmax."""
    import jax
    import jax.numpy as jnp

    cpu = jax.devices("cpu")[0]
    with jax.default_device(cpu):
        xj = jnp.asarray(xf)
        logits = (xj @ jnp.asarray(Wr)) / jnp.asarray(temp)
        topw, topi = jax.lax.top_k(logits, TOP_K)
        topw = jax.nn.softmax(topw, axis=-1)
    return np.asarray(topi), np.asarray(topw)


def _pad8(n):
    return max(P, ((n + 3) // 4) * 4)


def kernel(**inputs) -> np.ndarray:
    global LAST_RESULTS
    from concourse.bass_utils import run_bass_kernel_spmd

    x = np.asarray(inputs["x"], dtype=np.float32)
    Wr = np.asarray(inputs["Wr"], dtype=np.float32)
    temp = np.asarray(inputs["temp"], dtype=np.float32)
    W1 = np.asarray(inputs["W1"], dtype=np.float32)
    b1 = np.asarray(inputs["b1"], dtype=np.float32)
    W2 = np.asarray(inputs["W2"], dtype=np.float32)
    b2 = np.asarray(inputs["b2"], dtype=np.float32)

    B, S, D = x.shape
    T = B * S
    xf = x.reshape(T, D)

    topi, topw = _route_tokens(xf, Wr, temp)

    # Per-expert token lists and combine weights.
    tok_idx = []
    tok_w = []
    for e in range(NUM_EXPERTS):
        mask = topi == e                       # [T, K]
        sel = mask.any(axis=1)
        idx = np.nonzero(sel)[0]
        w = (topw * mask).sum(axis=1)[idx]
        tok_idx.append(idx)
        tok_w.append(w.astype(np.float32))

    counts = np.array([len(i) for i in tok_idx])
    order = np.argsort(-counts, kind="stable")
    # position p holds quarters of experts ranked 2p and 2p+1; core c runs
    # quarter c%4 of expert rank 2p + c//4 at position p
    pos_exp = [(order[2 * p], order[2 * p + 1]) for p in range(NSEC)]
    CS = tuple(_pad8(max(counts[ea], counts[eb])) for ea, eb in pos_exp)

    bf16 = ml_dtypes.bfloat16

    def xqT_of(e, p):
        C = CS[p]
        idx = tok_idx[e]
        xg = np.zeros((C, D), dtype=np.float32)
        xg[: len(idx)] = xf[idx]
        xg = xg.astype(bf16)
        # tile-major [P, K1*C]: per token tile a [P, K1, Nt] block
        outb = np.empty((P, K1 * C), dtype=bf16)
        for off, Nt in _tiles(C, first=(p == 0)):
            blk = xg[off:off + Nt].reshape(Nt, K1, P).transpose(2, 1, 0)
            outb[:, K1 * off:K1 * (off + Nt)] = blk.reshape(P, K1 * Nt)
        return outb

    xqT_cache = {}
    for p, (ea, eb) in enumerate(pos_exp):
        for e in (ea, eb):
            xqT_cache[e] = xqT_of(e, p)

    w1q = {}
    w2q = {}
    b1q = {}
    for e in range(NUM_EXPERTS):
        for q in range(NSEC):
            # chunk-major w1: [NBLK/2 i-chunks][K1][a|g][P cols]
            wq = np.empty((P, NBLK * K1 * P), dtype=bf16)
            W1e = W1[e]
            for i in range(K2):
                j = q * K2 + i
                acol = W1e[:, j * P:(j + 1) * P]              # [D, P]
                gcol = W1e[:, D_FF + j * P:D_FF + (j + 1) * P]
                # [D, 2P] -> [K1, P, 2P] -> [P, K1, 2P]
                blk = np.concatenate([acol, gcol], axis=1).astype(bf16)
                blk = blk.reshape(K1, P, 2 * P).transpose(1, 0, 2)
                wq[:, i * WCH:(i + 1) * WCH] = blk.reshape(P, WCH)
            w1q[(e, q)] = wq
            # W2[e][q*FQ:(q+1)*FQ, :]: [(K2 P), D] -> [P, K2*D]
            w2q[(e, q)] = np.ascontiguousarray(
                W2[e][q * FQ:(q + 1) * FQ, :].reshape(K2, P, D_MODEL)
                .transpose(1, 0, 2)).astype(bf16).reshape(P, K2 * D_MODEL)
            # b1 blocks: [a_0 g_0 a_1 g_1 ...] matching the i/a/g order
            cols = []
            for i in range(K2):
                j = q * K2 + i
                cols.append(np.arange(j * P, (j + 1) * P))
                cols.append(np.arange(D_FF + j * P, D_FF + (j + 1) * P))
            cols = np.concatenate(cols)
            b1q[(e, q)] = np.ascontiguousarray(
                b1[e][cols].reshape(NBLK, P).T)

    in_maps = []
    for c in range(N_CORES):
        q = c % 4
        m = {}
        for p, (ea, eb) in enumerate(pos_exp):
            e = ea if c < 4 else eb
            m[f"xq{p}"] = xqT_cache[e]
            m[f"w1{p}"] = w1q[(e, q)]
            m[f"w2{p}"] = w2q[(e, q)]
            m[f"b1t{p}"] = b1q[(e, q)]
        in_maps.append(m)

    if CS not in _NC_CACHE:
        _NC_CACHE[CS] = _build_nc(CS)
    nc = _NC_CACHE[CS]

    trace = bool(os.environ.get("MOE_KERNEL_TRACE"))
    kwargs = {}
    if trace:
        kwargs = dict(trace=True, trace_cores=list(range(N_CORES)))
    res = run_bass_kernel_spmd(nc, in_maps, core_ids=list(range(N_CORES)), **kwargs)
    LAST_RESULTS = res

    out = np.zeros((T, D), dtype=np.float32)
    for p, (ea, eb) in enumerate(pos_exp):
        tl = _tiles(CS[p], first=(p == 0))
        for h, e in enumerate((ea, eb)):
            idx = tok_idx[e]
            if len(idx) == 0:
                continue
            # sum the 4 quarter partials (cores 4h..4h+3, section p)
            yr = None
            for q in range(NSEC):
                yq = np.asarray(
                    res.results[4 * h + q][f"out{p}"]).astype(np.float32)
                yr = yq if yr is None else yr + yq
            # tile-major [P, NO*C] -> [C, D]
            y = np.empty((CS[p], D), dtype=np.float32)
            for off, Nt in tl:
                blk = yr[:, NO * off:NO * (off + Nt)].reshape(P, NO, Nt)
                y[off:off + Nt] = blk.transpose(2, 1, 0).reshape(Nt, D)
            y = y[: len(idx)] + b2[e]
            out[idx] += y * tok_w[e][:, None]

    return out.reshape(B, S, D)


# revision 27
# speedup vs baseline: 1.1840x; 1.1840x over previous
"""Trainium2 Bass kernel for AdaptiveMixtureOfExperts (top-2 SwiGLU MoE).

Strategy (expert-parallel, quarter-FF load balancing):
  - Host computes the tiny router (x @ Wr, top-2, softmax) with jax-on-CPU ops
    that bit-match the reference, then shards tokens by routed expert.
  - Each expert's FFN is split into 4 quarters along D_FF.  Experts are ranked
    by token count; rank pair (2p, 2p+1) forms section-position p, so each of
    the 8 cores runs 4 quarter-FFN sections (one per position).  Per-position
    token capacity = pad8(max count of the pair), giving per-core work within
    ~1.4% of the mean.
        hT = W1q.T @ xqT            (ff on partitions, tokens on free dim)
        uT = (a + b1a) * silu(g + b1g)
        yT_partial = W2q.T @ uT
  - Host sums the 4 quarter contributions per expert, adds b2, applies the
    top-2 combine weights, and scatter-adds into the full [B, S, D] output.

  DMA layout: every transfer is a contiguous 2D block with multi-KB rows
  (DMA descriptor feed is the early-phase bottleneck; thin-row strided DMAs
  feed ~4x slower).  Host packs token buffers tile-major ([P, K1*C], token
  tiles consecutive), W1 chunk-major ([P, NBLK/2 chunks of K1 x 2P cols]),
  and the y output tile-major ([P, NO*C]).  All inputs ride qSP in exact
  consumption order; y outputs and the tiny b1 vectors ride qACT so neither
  queue head-of-line blocks the other's dependencies.  Section 2/3 token
  loads (which wait on xg buffer reuse) precede their weights, idling the
  shared DMA pool exactly when mm2-s0/s1's outputs need to drain.

Shapes hardcoded for the problem instance:
  x:[2,2048,1024] f32, Wr:[1024,8], temp:[1], W1:[8,1024,4096], b1:[8,4096],
  W2:[8,2048,1024], b2:[8,1024].  TOP_K=2, 8 experts on 8 cores.
"""

import os

import numpy as np
import ml_dtypes

D_MODEL = 1024
D_FF = 2048
NUM_EXPERTS = 8
TOP_K = 2
P = 128          # partitions
NT = 512         # token tile (moving free dim per matmul)
T0 = 128         # first token tile of section 0 (small for an early start)
N_CORES = 8
NSEC = 4         # sections per core (one expert-quarter each)
FQ = D_FF // NSEC            # 512: ff quarter
K1 = D_MODEL // P            # 8 k-tiles for matmul1
K2 = FQ // P                 # 4 k-tiles for matmul2
NBLK = 2 * FQ // P           # 8 ff blocks per section (a/g interleaved)
NO = D_MODEL // P            # 8 output blocks of yT
WCH = K1 * 2 * P             # 2048: w1 columns per i-chunk in chunk-major
WARMUP = 13                  # PE warmup matmuls (cover preamble+first DMA)

_NC_CACHE = {}
LAST_RESULTS = None  # test harness introspection


def _tiles(C, first=False):
    """Token tile (offset, size) list.  With first=True the leading tile is
    halved so compute can start while the DMA ring is still slow; a 256-token
    tile's per-i compute (~1.7us) roughly matches one w1 chunk's DMA time."""
    cuts = [0]
    if first and 2 * T0 < C:
        cuts.append(2 * T0)
    while cuts[-1] < C:
        cuts.append(min(cuts[-1] + NT, C))
    return list(zip(cuts[:-1], (b - a for a, b in zip(cuts[:-1], cuts[1:]))))


def _build_nc(CS, use_silu: bool = True):
    """Per-core Bass graph: NSEC quarter-FF FFN sections of CS[s] tokens."""
    import concourse.mybir as mybir
    import concourse.tile as tile
    from concourse import bacc

    f32 = mybir.dt.float32
    bf16 = mybir.dt.bfloat16
    AF = mybir.ActivationFunctionType

    nc = bacc.Bacc()
    xq = {}
    w1 = {}
    w2 = {}
    b1t = {}
    outp = {}
    for s, C in enumerate(CS):
        xq[s] = nc.declare_dram_parameter(f"xq{s}", [P, K1 * C], bf16, isOutput=False)
        w1[s] = nc.declare_dram_parameter(f"w1{s}", [P, NBLK * K1 * P], bf16, isOutput=False)
        w2[s] = nc.declare_dram_parameter(f"w2{s}", [P, K2 * D_MODEL], bf16, isOutput=False)
        b1t[s] = nc.declare_dram_parameter(f"b1t{s}", [P, NBLK], f32, isOutput=False)
        # partial y without b2 (host adds the bias once per expert), bf16 to
        # halve output DMA bytes; tile-major [P, NO*C]
        outp[s] = nc.declare_dram_parameter(f"out{s}", [P, NO * C], bf16, isOutput=True)

    CMAX = max(CS)

    with tile.TileContext(nc) as tc:
        with (
            tc.tile_pool(name="weights", bufs=1) as wpool,
            tc.tile_pool(name="acts", bufs=2) as upool,
            tc.tile_pool(name="epilogue", bufs=4) as epool,
            tc.tile_pool(name="ps", bufs=8, space="PSUM") as ps_pool,
        ):
            b1_sb = {}
            for s in range(NSEC):
                b1_sb[s] = wpool.tile([P, NBLK], f32, name=f"b1_sb{s}", tag=f"b1{s}")

            # weights resident per section; tokens double-buffered across
            # sections (section s+2's load waits for section s's reads)
            w1_sb = {}
            w2_sb = {}
            xg_sb = {}
            for s in range(NSEC):
                w1_sb[s] = wpool.tile([P, NBLK * K1 * P], bf16, name=f"w1_sb{s}",
                                      tag=f"w1{s}")
                w2_sb[s] = wpool.tile([P, K2 * D_MODEL], bf16, name=f"w2_sb{s}",
                                      tag=f"w2{s}")
                xg_sb[s] = upool.tile([P, K1 * CMAX], bf16, name=f"xg_sb{s}",
                                      tag="xg", bufs=2)

            # PE warmup: dummy matmuls on a zeroed tile keep the PE busy (and
            # open the HAM clock gate to 2.4 GHz) until the first input DMAs
            # land (~6us fixed preamble + first chunks).
            warm = wpool.tile([P, NT], bf16, name="warm")
            nc.gpsimd.memset(warm[:], 0.0)
            ps_w = ps_pool.tile([P, NT], f32, name="ps_warm", tag="ps")
            for _ in range(WARMUP):
                nc.tensor.matmul(ps_w[:], warm[:, :P], warm[:], start=True, stop=True)
            # gate matmuls: read (uninitialized) section-1 tiles so their
            # DMAs carry a write-after-read wait until warmup completes.
            # With all 8 cores hammering the shared HBM path at launch, this
            # halves the aggregate early demand (only section-0 streams),
            # so every core's critical first chunks land sooner.  The values
            # read are garbage into a dead PSUM tile -- never consumed.
            nc.tensor.matmul(ps_w[:], w1_sb[1][:, :P], xg_sb[1][:, :NT],
                             start=True, stop=True)
            nc.tensor.matmul(ps_w[:], w2_sb[1][:, :P], warm[:],
                             start=True, stop=True)

            # ---- tiny early inputs on qACT (biases) ----
            for s in range(NSEC):
                nc.scalar.dma_start(out=b1_sb[s][:], in_=b1t[s][:])

            # ---- bulk inputs on qSP in exact PE consumption order ----
            # (the shared DMA pool caps at ~220GB/s; both queues draw from
            # it, so the early phase is paced purely by bytes-before-need)
            def emit_w1(s, i0, i1):
                nc.sync.dma_start(
                    out=w1_sb[s][:, i0 * WCH:i1 * WCH],
                    in_=w1[s][:, i0 * WCH:i1 * WCH])

            def emit_xg(s, off, end):
                nc.sync.dma_start(
                    out=xg_sb[s][:, K1 * off:K1 * end],
                    in_=xq[s][:, K1 * off:K1 * end])

            # section 0: mm1 runs t-outer, so tile 0 needs w1 i-chunks in
            # order (0.5MB steps) and later tiles need tokens -- emit in that
            # exact consumption order so each wait is short and the PE never
            # idles past the HAM re-throttle window
            s0_tiles = _tiles(CS[0], first=True)
            emit_w1(0, 0, 1)
            emit_xg(0, 0, s0_tiles[0][1])
            emit_w1(0, 1, 2)
            emit_xg(0, s0_tiles[1][0], s0_tiles[1][0] + s0_tiles[1][1])
            emit_w1(0, 2, 3)
            emit_w1(0, 3, K2)
            for off, Nt in s0_tiles[2:]:
                emit_xg(0, off, off + Nt)
            nc.sync.dma_start(out=w2_sb[0][:], in_=w2[0][:])
            # section 1: w1 head, tokens, w1 tail, w2
            emit_w1(1, 0, 1)
            emit_xg(1, 0, CS[1])
            emit_w1(1, 1, K2)
            nc.sync.dma_start(out=w2_sb[1][:], in_=w2[1][:])
            # sections 2/3: token loads reuse the xg buffers of sections 0/1,
            # so they wait on mm1-s0/s1's last reads.  Putting them FIRST
            # deliberately head-of-line blocks the input queue at ~the end of
            # mm1-s0/s1 -- that idles the shared DMA-engine pool exactly when
            # the y outputs of mm2-s0/s1 need it, then resumes with s2/s3
            # weights (still >30us before their matmuls need them)
            emit_xg(2, 0, CS[2])
            emit_w1(2, 0, K2)
            nc.sync.dma_start(out=w2_sb[2][:], in_=w2[2][:])
            emit_xg(3, 0, CS[3])
            emit_w1(3, 0, K2)
            nc.sync.dma_start(out=w2_sb[3][:], in_=w2[3][:])

            # ---- main loops ----
            # w1_sb chunk-major: i-block i at [i*WCH, (i+1)*WCH), inside it
            # k at [k*2P, k*2P+2P), a then g.  xg_sb tile-major: token tile
            # at [K1*off, K1*(off+Nt)), inside it k at [k*Nt, (k+1)*Nt).
            uT = {}

            def emit_mm1(s, staggered=False):
                tl = _tiles(CS[s], first=staggered)
                for t in range(len(tl)):
                    uT[(s, t)] = upool.tile(
                        [P, K2, NT], bf16, name=f"uT{s}{t}", tag="uT", bufs=4)
                if staggered and len(tl) > 2:
                    # consume in exact DMA-arrival order (queue: w1c0, xgt0,
                    # w1c1, xgt1, w1c2, w1c3, xgt2...): zig-zag t0/t1 so no
                    # single data-wait exceeds the ~3.4us HAM re-throttle
                    # window on a slow-ring core
                    sched = [(0, 0), (0, 1), (1, 0), (1, 1),
                             (0, 2), (1, 2), (0, 3), (1, 3)]
                    sched += [(t, i) for t in range(2, len(tl))
                              for i in range(K2)]
                else:
                    sched = [(t, i) for t in range(len(tl))
                             for i in range(K2)]
                for t, i in sched:
                        off, Nt = tl[t]
                        xbase = K1 * off
                        wbase = i * WCH
                        ps_a = ps_pool.tile(
                            [P, NT], f32, name=f"psa{s}{t}_{i}", tag="ps")
                        for k in range(K1):
                            nc.tensor.matmul(
                                ps_a[:, :Nt],
                                w1_sb[s][:, wbase + k * 2 * P:wbase + k * 2 * P + P],
                                xg_sb[s][:, xbase + k * Nt:xbase + (k + 1) * Nt],
                                start=(k == 0),
                                stop=(k == K1 - 1),
                            )
                        ps_g = ps_pool.tile(
                            [P, NT], f32, name=f"psg{s}{t}_{i}", tag="ps")
                        for k in range(K1):
                            nc.tensor.matmul(
                                ps_g[:, :Nt],
                                w1_sb[s][:, wbase + k * 2 * P + P:wbase + (k + 1) * 2 * P],
                                xg_sb[s][:, xbase + k * Nt:xbase + (k + 1) * Nt],
                                start=(k == 0),
                                stop=(k == K1 - 1),
                            )
                        a_t = epool.tile([P, NT], bf16, name=f"a{s}{t}_{i}",
                                         tag="a")
                        nc.scalar.activation(
                            a_t[:, :Nt], ps_a[:, :Nt], AF.Identity,
                            bias=b1_sb[s][:, 2 * i:2 * i + 1],
                        )
                        g_t = epool.tile([P, NT], bf16, name=f"g{s}{t}_{i}",
                                         tag="g")
                        if use_silu:
                            nc.scalar.activation(
                                g_t[:, :Nt], ps_g[:, :Nt], AF.Silu,
                                bias=b1_sb[s][:, 2 * i + 1:2 * i + 2],
                            )
                        else:
                            s_t = epool.tile(
                                [P, NT], bf16, name=f"s{s}{t}_{i}", tag="s")
                            nc.scalar.activation(
                                s_t[:, :Nt], ps_g[:, :Nt], AF.Sigmoid,
                                bias=b1_sb[s][:, 2 * i + 1:2 * i + 2],
                            )
                            gb_t = epool.tile(
                                [P, NT], bf16, name=f"gb{s}{t}_{i}", tag="gb")
                            nc.scalar.activation(
                                gb_t[:, :Nt], ps_g[:, :Nt], AF.Identity,
                                bias=b1_sb[s][:, 2 * i + 1:2 * i + 2],
                            )
                            nc.vector.tensor_mul(
                                g_t[:, :Nt], gb_t[:, :Nt], s_t[:, :Nt])
                        nc.vector.tensor_mul(
                            uT[(s, t)][:, i, :Nt], a_t[:, :Nt], g_t[:, :Nt])

            def emit_mm2(s, staggered=False, fine_tail=False):
                tl = _tiles(CS[s], first=staggered)
                for t, (off, Nt) in enumerate(tl):
                    last_tile = fine_tail and t == len(tl) - 1
                    ybase = NO * off
                    y_t = epool.tile([P, NO * NT], bf16, name=f"y{s}{t}",
                                     tag="y", bufs=4)
                    for m in range(NO):
                        ps_y = ps_pool.tile(
                            [P, NT], f32, name=f"psy{s}{t}_{m}", tag="ps")
                        for k in range(K2):
                            nc.tensor.matmul(
                                ps_y[:, :Nt],
                                w2_sb[s][:, k * D_MODEL + m * P:k * D_MODEL + (m + 1) * P],
                                uT[(s, t)][:, k, :Nt],
                                start=(k == 0),
                                stop=(k == K2 - 1),
                            )
                        # psum drain on DVE (idle), output via qACT (y DMAs +
                        # b1 are the only users, so the input stream on qSP is
                        # never blocked and y drains promptly)
                        nc.vector.tensor_copy(y_t[:, m * Nt:(m + 1) * Nt],
                                              ps_y[:, :Nt])
                        if last_tile:
                            nc.scalar.dma_start(
                                out=outp[s][:, ybase + m * Nt:ybase + (m + 1) * Nt],
                                in_=y_t[:, m * Nt:(m + 1) * Nt],
                            )
                        elif m == NO // 2 - 1:
                            nc.scalar.dma_start(
                                out=outp[s][:, ybase:ybase + (NO // 2) * Nt],
                                in_=y_t[:, :(NO // 2) * Nt],
                            )
                        elif m == NO - 1:
                            nc.scalar.dma_start(
                                out=outp[s][:, ybase + (NO // 2) * Nt:ybase + NO * Nt],
                                in_=y_t[:, (NO // 2) * Nt:NO * Nt],
                            )

            for s in range(NSEC):
                emit_mm1(s, staggered=(s == 0))
                emit_mm2(s, staggered=(s == 0), fine_tail=(s == NSEC - 1))

    nc.compile()
    return nc


def _route_tokens(xf, Wr, temp):
    """Bit-match the reference's router on CPU jax: logits, top-2, softmax."""
    import jax
    import jax.numpy as jnp

    cpu = jax.devices("cpu")[0]
    with jax.default_device(cpu):
        xj = jnp.asarray(xf)
        logits = (xj @ jnp.asarray(Wr)) / jnp.asarray(temp)
        topw, topi = jax.lax.top_k(logits, TOP_K)
        topw = jax.nn.softmax(topw, axis=-1)
    return np.asarray(topi), np.asarray(topw)


def _pad8(n):
    return max(P, ((n + 3) // 4) * 4)


def kernel(**inputs) -> np.ndarray:
    global LAST_RESULTS
    from concourse.bass_utils import run_bass_kernel_spmd

    x = np.asarray(inputs["x"], dtype=np.float32)
    Wr = np.asarray(inputs["Wr"], dtype=np.float32)
    temp = np.asarray(inputs["temp"], dtype=np.float32)
    W1 = np.asarray(inputs["W1"], dtype=np.float32)
    b1 = np.asarray(inputs["b1"], dtype=np.float32)
    W2 = np.asarray(inputs["W2"], dtype=np.float32)
    b2 = np.asarray(inputs["b2"], dtype=np.float32)

    B, S, D = x.shape
    T = B * S
    xf = x.reshape(T, D)

    topi, topw = _route_tokens(xf, Wr, temp)

    # Per-expert token lists and combine weights.
    tok_idx = []
    tok_w = []
    for e in range(NUM_EXPERTS):
        mask = topi == e                       # [T, K]
        sel = mask.any(axis=1)
        idx = np.nonzero(sel)[0]
        w = (topw * mask).sum(axis=1)[idx]
        tok_idx.append(idx)
        tok_w.append(w.astype(np.float32))

    counts = np.array([len(i) for i in tok_idx])
    order = np.argsort(-counts, kind="stable")
    # position p holds quarters of experts ranked 2p and 2p+1; core c runs
    # quarter c%4 of expert rank 2p + c//4 at position p
    pos_exp = [(order[2 * p], order[2 * p + 1]) for p in range(NSEC)]
    CS = tuple(_pad8(max(counts[ea], counts[eb])) for ea, eb in pos_exp)

    bf16 = ml_dtypes.bfloat16

    def xqT_of(e, p):
        C = CS[p]
        idx = tok_idx[e]
        xg = np.zeros((C, D), dtype=np.float32)
        xg[: len(idx)] = xf[idx]
        xg = xg.astype(bf16)
        # tile-major [P, K1*C]: per token tile a [P, K1, Nt] block
        outb = np.empty((P, K1 * C), dtype=bf16)
        for off, Nt in _tiles(C, first=(p == 0)):
            blk = xg[off:off + Nt].reshape(Nt, K1, P).transpose(2, 1, 0)
            outb[:, K1 * off:K1 * (off + Nt)] = blk.reshape(P, K1 * Nt)
        return outb

    xqT_cache = {}
    for p, (ea, eb) in enumerate(pos_exp):
        for e in (ea, eb):
            xqT_cache[e] = xqT_of(e, p)

    w1q = {}
    w2q = {}
    b1q = {}
    for e in range(NUM_EXPERTS):
        for q in range(NSEC):
            # chunk-major w1: [NBLK/2 i-chunks][K1][a|g][P cols]
            wq = np.empty((P, NBLK * K1 * P), dtype=bf16)
            W1e = W1[e]
            for i in range(K2):
                j = q * K2 + i
                acol = W1e[:, j * P:(j + 1) * P]              # [D, P]
                gcol = W1e[:, D_FF + j * P:D_FF + (j + 1) * P]
                # [D, 2P] -> [K1, P, 2P] -> [P, K1, 2P]
                blk = np.concatenate([acol, gcol], axis=1).astype(bf16)
                blk = blk.reshape(K1, P, 2 * P).transpose(1, 0, 2)
                wq[:, i * WCH:(i + 1) * WCH] = blk.reshape(P, WCH)
            w1q[(e, q)] = wq
            # W2[e][q*FQ:(q+1)*FQ, :]: [(K2 P), D] -> [P, K2*D]
            w2q[(e, q)] = np.ascontiguousarray(
                W2[e][q * FQ:(q + 1) * FQ, :].reshape(K2, P, D_MODEL)
                .transpose(1, 0, 2)).astype(bf16).reshape(P, K2 * D_MODEL)
            # b1 blocks: [a_0 g_0 a_1 g_1 ...] matching the i/a/g order
            cols = []
            for i in range(K2):
                j = q * K2 + i
                cols.append(np.arange(j * P, (j + 1) * P))
                cols.append(np.arange(D_FF + j * P, D_FF + (j + 1) * P))
            cols = np.concatenate(cols)
            b1q[(e, q)] = np.ascontiguousarray(
                b1[e][cols].reshape(NBLK, P).T)

    in_maps = []
    for c in range(N_CORES):
        q = c % 4
        m = {}
        for p, (ea, eb) in enumerate(pos_exp):
            e = ea if c < 4 else eb
            m[f"xq{p}"] = xqT_cache[e]
            m[f"w1{p}"] = w1q[(e, q)]
            m[f"w2{p}"] = w2q[(e, q)]
            m[f"b1t{p}"] = b1q[(e, q)]
        in_maps.append(m)

    if CS not in _NC_CACHE:
        _NC_CACHE[CS] = _build_nc(CS)
    nc = _NC_CACHE[CS]

    trace = bool(os.environ.get("MOE_KERNEL_TRACE"))
    kwargs = {}
    if trace:
        kwargs = dict(trace=True, trace_cores=list(range(N_CORES)))
    res = run_bass_kernel_spmd(nc, in_maps, core_ids=list(range(N_CORES)), **kwargs)
    LAST_RESULTS = res

    out = np.zeros((T, D), dtype=np.float32)
    for p, (ea, eb) in enumerate(pos_exp):
        tl = _tiles(CS[p], first=(p == 0))
        for h, e in enumerate((ea, eb)):
            idx = tok_idx[e]
            if len(idx) == 0:
                continue
            # sum the 4 quarter partials (cores 4h..4h+3, section p)
            yr = None
            for q in range(NSEC):
                yq = np.asarray(
                    res.results[4 * h + q][f"out{p}"]).astype(np.float32)
                yr = yq if yr is None else yr + yq
            # tile-major [P, NO*C] -> [C, D]
            y = np.empty((CS[p], D), dtype=np.float32)
            for off, Nt in tl:
                blk = yr[:, NO * off:NO * (off + Nt)].reshape(P, NO, Nt)
                y[off:off + Nt] = blk.transpose(2, 1, 0).reshape(Nt, D)
            y = y[: len(idx)] + b2[e]
            out[idx] += y * tok_w[e][:, None]

    return out.reshape(B, S, D)
